# revision 37
# baseline (speedup 1.0000x reference)
"""Multi-head attention (B=2, L=2048, D=1024, H=16) on 8 trn2 cores.

Sharding: core c -> (batch b = c//4) x (head-group hg = c%4, 4 heads each).
W_q/W_k/W_v are column-split, W_o row-split; the 4 partial outputs per
batch are summed on the host (plus bo).  Masked keys are dropped on the
host (key compaction); pad slots are neutralized by zeroing the
denominator-ones column of v_all, so exp needs no mask bias at all.

Precision plan (rel-err budget 2e-2, measured ~6e-3):
  - Q/K/V projections run as fp8e4m3 hi/lo x hi/lo DoubleRow chains
    (3 chains: Whi*xhi + Whi*xlo + Wlo*xhi).  Each value is split as
    v = hi + lo with hi = fp8(v*scale), lo = fp8(v*scale - hi); the
    product error is ~0.13% -- better than bf16 -- at 3/4 of the bf16
    PE cycles (DoubleRow contracts 2 dc-chunks per instr at 0.5 cyc/row).
  - Scores and attn@V stay bf16 (single-sided fp8 measured over-budget).
  - attn@V is computed FLIPPED: out[128 Lq, 65] with P as the stationary
    operand and v_all (64 V cols + ones col) moving, so each matmul costs
    N=65 cycles instead of 512 -- attn@V PE time halves, and the softmax
    denominator lands on the free axis: normalize is a per-partition
    reciprocal + tensor_scalar, no partition broadcast needed.
  - The normalized [128 Lq, 4t x 2h x 64d] tiles are transposed back to
    [C, Lq] for the output projection by one blocked transpose DMA per
    wave ([128,512] -> [128,4,128], out[p,j,f] = in[f,128j+p]).

Engine budget: ACT runs one paired exp per (wave, c) over a 2-bank psum
tile [128, 2(heads), 512] (free=1024 amortizes the ACT access latency);
PE runs matmuls; DVE does evacuations + normalize; Pool evacuates the
output projection.  Software pipeline: wave (lb, g)'s phase A (scores +
exp) absorbs as background ops the previous wave's phase B (attn@V +
normalize + transpose), projection tails, and the output projection of
bank lb-1, paced by the ACT exp stream.
"""

import math
import sys

for _p in ("/opt/trn_rl_repo",):
    if _p not in sys.path:
        sys.path.insert(0, _p)

import numpy as np

import concourse.bass as bass
import concourse.mybir as mybir
import concourse.tile as tile
from concourse import bacc
from concourse.bass import ts
from concourse.bass_utils import run_bass_kernel_spmd
from concourse.tile_rust import add_dep_helper

F32 = mybir.dt.float32
BF16 = mybir.dt.bfloat16
F8 = mybir.dt.float8e4

D_MODEL = 1024
NUM_HEADS = 16
D_K = 64
B = 2
L = 2048
N_CORES = 8
HPC = NUM_HEADS // 4  # heads per core (4)
C = HPC * D_K         # attention columns per core (256)
CT = C // 128         # col tile groups (2)
DC = D_MODEL // 128   # d_model chunks (8)
LB = L // 512         # Lq banks (4)
SCALE = float(np.sqrt(D_K))

SW = 1024.0           # host pre-scale on W before fp8 hi/lo split
SX = 16.0             # host pre-scale on x before fp8 hi/lo split
PS_SCALE = 1.0 / (SW * SX)   # psum -> true units
SCORES_FP8 = False
SKQ = 32.0            # K/Q fp8 scale (fp8 scores path)
DR = mybir.MatmulPerfMode.DoubleRow


def build_nc(LkP, L=L, D=D_MODEL, no_bias=True, scores_fp8=SCORES_FP8):
    """Per-core Bass program (SPMD, 8 cores) for LkP compacted keys."""
    DK = D_K
    LTk = LkP // 128          # key tiles
    KB = (LkP + 511) // 512   # xk/xv 512-wide load blocks
    NC2 = DC // 2             # DoubleRow dc-pairs (4)

    nc = bacc.Bacc("TRN2", target_bir_lowering=False, debug=False,
                   num_devices=N_CORES)

    x_in = {}
    for n, wid in (("q", L), ("k", LkP), ("v", LkP)):
        for hl in ("h", "l"):
            x_in[n, hl] = nc.dram_tensor(
                f"x{n}{hl}", [D, wid], F8, kind="ExternalInput").ap()
    w_in = {}
    for n in ("q", "k", "v"):
        for hl in ("h", "l"):
            # host pre-packs to the SBUF layout: rows of DC*C=2048 bytes
            # (a [D, C] layout would DMA 256-byte rows at half bus rate)
            w_in[n, hl] = nc.dram_tensor(
                f"w{n}{hl}", [128, DC * C], F8, kind="ExternalInput").ap()
    wo = nc.dram_tensor("wo", [C, D], BF16, kind="ExternalInput").ap()
    vones = nc.dram_tensor("vones", [128, LTk], BF16,
                           kind="ExternalInput").ap()
    if not no_bias:
        bias = {n: nc.dram_tensor(f"b{n}", [C], F32,
                                  kind="ExternalInput").ap()
                for n in ("q", "k")}
        bvr = nc.dram_tensor("bvr", [1, C], BF16, kind="ExternalInput").ap()
    out = nc.dram_tensor("partial", [L, D], BF16, kind="ExternalOutput").ap()

    with tile.TileContext(nc) as tc:
        with (
            tc.tile_pool(name="consts", bufs=1) as consts,
            tc.tile_pool(name="persist", bufs=1) as persist,
            tc.tile_pool(name="xch", bufs=4) as xch,
            tc.tile_pool(name="pp", bufs=20) as ppool,
            tc.tile_pool(name="astg", bufs=3) as astgp,
            tc.tile_pool(name="norm", bufs=8) as normp,
            tc.tile_pool(name="ostg", bufs=8) as ostgp,
            tc.tile_pool(name="ps2", bufs=2, space="PSUM") as ps2p,
            tc.tile_pool(name="psa", bufs=2, space="PSUM") as psap,
            tc.tile_pool(name="psw", bufs=2, space="PSUM") as pswp,
        ):
            # ---- DMA rings: weights + x blocks in deadline order.
            # Two HWDGE queues: SP carries the K/V streams, the ACT ring
            # carries the Q stream so the prologue's critical DMA path is
            # split across two queues (ACT's ring is drained before the
            # first exp issues, so it never blocks the activation stream).
            w_sb, b_sb = {}, {}
            last_dma = [None]
            last_dma2 = [None]

            def chain(xd):
                if last_dma[0] is not None:
                    add_dep_helper(xd.ins, last_dma[0].ins, sync=False,
                                   reason="dma-order")
                last_dma[0] = xd

            def chain2(xd):
                if last_dma2[0] is not None:
                    add_dep_helper(xd.ins, last_dma2[0].ins, sync=False,
                                   reason="dma2-order")
                last_dma2[0] = xd

            def load_w(n, hl, eng=None):
                w_sb[n, hl] = consts.tile([128, DC, C], F8, tag=f"w{n}{hl}",
                                          name=f"w{n}{hl}_sb")
                ch, e = ((chain2, nc.scalar) if eng == "act"
                         else (chain, nc.sync))
                ch(e.dma_start(
                    out=w_sb[n, hl],
                    in_=w_in[n, hl].rearrange("p (c n) -> p c n", c=DC)))

            def load_b(n):
                b_sb[n] = consts.tile([128, CT], F32, tag=f"b{n}",
                                      name=f"b{n}_sb")
                nc.gpsimd.dma_start(
                    out=b_sb[n], in_=bias[n].rearrange("(t p) -> p t", p=128))

            if not no_bias:
                bvr_sb = consts.tile([1, C], BF16, tag="bvr")
                nc.gpsimd.dma_start(out=bvr_sb, in_=bvr)
                load_b("k")
                load_b("q")
                ones_sb = consts.tile([1, 128], BF16, tag="ones")
                nc.vector.memset(ones_sb, 1.0)
            vones_sb = consts.tile([128, LTk], BF16, tag="vones")
            nc.gpsimd.dma_start(out=vones_sb, in_=vones)

            v_all = persist.tile([128, LTk, HPC, DK + 1], BF16, tag="vall")
            # denominator-ones column; zero at pad rows via per-core data
            nc.vector.tensor_copy(
                out=v_all[:, :, :, DK],
                in_=vones_sb.unsqueeze(2).broadcast_to([128, LTk, HPC]))

            def xload_dc(n, hl, j, wdt, eng=None):
                # dc-split load: two tiles [128, DC/2, wdt] so projection
                # chains start after half the bytes (rows stay 512B-wide;
                # column-splits would halve the DMA bus rate)
                ch, e = ((chain2, nc.scalar) if eng == "act"
                         else (chain, nc.sync))
                pair = []
                for half in range(2):
                    t = xch.tile([128, DC // 2, 512], F8, tag="xbh",
                                 bufs=8, name="xbh")
                    ch(e.dma_start(
                        out=t[:, :, :wdt],
                        in_=x_in[n, hl][:, j * 512:j * 512 + wdt].rearrange(
                            "(c p) n -> p c n", p=128)
                        [:, half * (DC // 2):(half + 1) * (DC // 2), :]))
                    pair.append(t)
                return pair

            def xload(n, hl, j, wdt, o=0, xb=None, eng=None):
                if xb is None:
                    xb = xch.tile([128, DC, 512], F8, tag="xb", bufs=8,
                                  name="xb")
                ch, e = ((chain2, nc.scalar) if eng == "act"
                         else (chain, nc.sync))
                ch(e.dma_start(
                    out=xb[:, :, o:o + wdt],
                    in_=x_in[n, hl][:, j * 512 + o:j * 512 + o + wdt]
                    .rearrange("(c p) n -> p c n", p=128)))
                return xb

            kw = [min(512, LkP - j * 512) for j in range(KB)]
            xkb, xvb, xqb = {}, {}, {}
            load_w("k", "h")
            load_w("k", "l")
            xk0 = {hl: xload_dc("k", hl, 0, kw[0]) for hl in ("h", "l")}
            load_w("q", "h", eng="act")
            load_w("q", "l", eng="act")
            xq0 = {hl: xload_dc("q", hl, 0, 512, eng="act")
                   for hl in ("h", "l")}
            # hold the K tail blocks behind the prologue-critical xq0
            # stream (cross-ring order: one global DMA engine pool)
            add_dep_helper_xq0 = last_dma2[0]
            first_tail = [True]

            def hold_xk_tail(xd):
                if first_tail[0] and add_dep_helper_xq0 is not None:
                    add_dep_helper(xd.ins, add_dep_helper_xq0.ins,
                                   sync=False, reason="xk-after-xq0")
                    first_tail[0] = False
            for j in range(1, KB):
                xkb[j, "h"] = xload("k", "h", j, kw[j])
                hold_xk_tail(last_dma[0])
                xkb[j, "l"] = xload("k", "l", j, kw[j])
            load_w("v", "h")
            load_w("v", "l")
            for j in range(KB):
                xvb[j, "h"] = xload("v", "h", j, kw[j])
                xvb[j, "l"] = xload("v", "l", j, kw[j])
            xqb[1, "h"] = xload("q", "h", 1, 512)
            xqb[1, "l"] = xload("q", "l", 1, 512)
            wo_sb = consts.tile([128, CT, D], BF16, tag="wo")
            chain(nc.sync.dma_start(
                out=wo_sb, in_=wo.rearrange("(g p) n -> p g n", p=128)))
            for j in range(2, LB):
                xqb[j, "h"] = xload("q", "h", j, 512)
                xqb[j, "l"] = xload("q", "l", j, 512)

            if scores_fp8:
                kt8 = persist.tile([128, CT, LTk, 2, 128], F8, tag="kt8")
                qt8 = persist.tile([128, CT, L], F8, tag="qt8")
            else:
                KT = persist.tile([128, CT, LkP], BF16, tag="kt")
                QT = persist.tile([128, CT, L], BF16, tag="qt")
            ot_sb = persist.tile([128, CT, L], BF16, tag="ot")

            # ---- hi/lo DoubleRow projection chains ----
            CHAINS = (("h", "h"), ("h", "l"), ("l", "h"))

            def proj_group(ps, wname, xt, wdt, gs, due=None):
                """12 DR matmuls accumulating W.T @ x into ps[:, :wdt].
                xt maps hl -> x tile AP sliced [128, DC-pair, wdt]."""
                ops = []
                for ci, (wl, xl) in enumerate(CHAINS):
                    for i in range(NC2):
                        def op_mm(ci=ci, i=i, wl=wl, xl=xl):
                            nc.tensor.matmul(
                                ps[:, :wdt],
                                lhsT=w_sb[wname, wl][:, 2 * i:2 * i + 2, gs],
                                rhs=xt(xl, i),
                                start=(ci == 0 and i == 0),
                                stop=(ci == 2 and i == NC2 - 1),
                                perf_mode=DR)
                        ops.append((wdt * 0.5 * 0.4167, due, None, op_mm))
                return ops

            def kproj_ops(j, o=0, wdt=None, due=None,
                          groups=range(CT)):
                if wdt is None:
                    wdt = kw[j] - o
                ops = []
                for g in groups:
                    ps = pswp.tile([128, 512], F32, tag="psw", name="kps")

                    def xt(xl, i, j=j, o=o, wdt=wdt):
                        if j == 0:
                            t = xk0[xl][i // 2]
                            return t[:, 2 * (i % 2):2 * (i % 2) + 2,
                                     o:o + wdt]
                        return xkb[j, xl][:, 2 * i:2 * i + 2, o:o + wdt]

                    ops += proj_group(ps, "k", xt, wdt, ts(g, 128),
                                      due=due)

                    def op_ev(ps=ps, j=j, g=g, o=o, wdt=wdt):
                        sl = slice(j * 512 + o, j * 512 + o + wdt)
                        if scores_fp8:
                            nt = wdt // 128
                            c0 = (j * 512 + o) // 128
                            ps3 = ps[:, :wdt].rearrange(
                                "p (a b) -> p a b", b=128)
                            hi = kt8[:, g, c0:c0 + nt, 0, :]
                            if no_bias:
                                nc.vector.tensor_scalar_mul(
                                    hi, ps3, SKQ * PS_SCALE)
                            else:
                                nc.vector.tensor_scalar(
                                    hi, ps3, SKQ * PS_SCALE,
                                    b_sb["k"][:, g:g + 1],
                                    mybir.AluOpType.mult,
                                    mybir.AluOpType.add)
                            nc.vector.scalar_tensor_tensor(
                                out=kt8[:, g, c0:c0 + nt, 1, :],
                                in0=ps3, scalar=SKQ * PS_SCALE,
                                in1=hi,
                                op0=mybir.AluOpType.mult,
                                op1=mybir.AluOpType.subtract)
                        elif no_bias:
                            nc.vector.tensor_scalar_mul(
                                KT[:, g, sl], ps[:, :wdt], PS_SCALE)
                        else:
                            nc.vector.tensor_scalar(
                                KT[:, g, sl], ps[:, :wdt], PS_SCALE,
                                b_sb["k"][:, g:g + 1],
                                mybir.AluOpType.mult, mybir.AluOpType.add)
                    ops.append((0, due, None, op_ev))
                return ops

            def qproj_ops(lb, due=None, groups=range(CT), o=0, wdt=512):
                ops = []
                for g in groups:
                    ps = pswp.tile([128, 512], F32, tag="psw", name="qps")

                    def xt(xl, i, lb=lb, o=o, wdt=wdt):
                        if lb == 0:
                            t = xq0[xl][i // 2]
                            return t[:, 2 * (i % 2):2 * (i % 2) + 2,
                                     o:o + wdt]
                        return xqb[lb, xl][:, 2 * i:2 * i + 2, o:o + wdt]

                    ops += proj_group(ps, "q", xt, wdt, ts(g, 128),
                                      due=due)

                    def op_qe(ps=ps, lb=lb, g=g, o=o, wdt=wdt):
                        sl = slice(lb * 512 + o, lb * 512 + o + wdt)
                        if scores_fp8:
                            dst = qt8[:, g, sl]
                            sc = SKQ * PS_SCALE
                        else:
                            dst = QT[:, g, sl]
                            sc = PS_SCALE
                        if no_bias:
                            nc.vector.tensor_scalar_mul(
                                dst, ps[:, :wdt], sc)
                        else:
                            nc.vector.tensor_scalar(
                                dst, ps[:, :wdt], sc, b_sb["q"][:, g:g + 1],
                                mybir.AluOpType.mult, mybir.AluOpType.add)
                    ops.append((0, due, None, op_qe))
                return ops

            def vproj_ops(c0, c1, hold=None):
                # x is stationary, W moving: out[128 Lk, C]
                ops = []
                for c in range(c0, c1):
                    j, o = c // 4, (c % 4) * 128
                    ps = pswp.tile([128, 512], F32, tag="psw", name="vps")
                    opn = []
                    if not no_bias:
                        def op_b(ps=ps):
                            nc.tensor.matmul(
                                ps[:, :C], lhsT=ones_sb, rhs=bvr_sb,
                                start=True, stop=False)
                        opn = [(C * 0.4167, None, hold, op_b)]
                    for ci, (xl, wl) in enumerate(CHAINS):
                        for i in range(NC2):
                            def op_mm(ci=ci, i=i, xl=xl, wl=wl, j=j, o=o,
                                      ps=ps):
                                nc.tensor.matmul(
                                    ps[:, :C],
                                    lhsT=xvb[j, xl][:, 2 * i:2 * i + 2,
                                                    o:o + 128],
                                    rhs=w_sb["v", wl][:, 2 * i:2 * i + 2, :],
                                    start=(no_bias and ci == 0 and i == 0),
                                    stop=(ci == 2 and i == NC2 - 1),
                                    perf_mode=DR)
                            opn.append((C * 0.5 * 0.4167, None, hold, op_mm))
                    ops += opn

                    def op_ev(ps=ps, c=c):
                        nc.vector.tensor_scalar_mul(
                            v_all[:, c, :, 0:DK],
                            ps[:, :C].rearrange("p (h d) -> p h d", h=HPC),
                            PS_SCALE)
                    ops.append((0, None, None, op_ev))
                return ops

            def oproj_ops(lb, swdge_only=False):
                ops = []
                for tt in range(4):
                    t = lb * 4 + tt
                    for half in range(2):
                        wps = pswp.tile([128, 512], F32, tag="psw",
                                        name="wps")
                        for g in range(CT):
                            ops.append((512 * 0.4167, None, None,
                                        lambda wps=wps, t=t, half=half, g=g:
                                        nc.tensor.matmul(
                                            wps,
                                            lhsT=ot_sb[:, g, ts(t, 128)],
                                            rhs=wo_sb[:, g, ts(half, 512)],
                                            start=(g == 0),
                                            stop=(g == CT - 1))))

                        def op_stage(wps=wps, t=t, half=half,
                                     swdge_only=swdge_only):
                            og = ostgp.tile([128, 512], BF16, tag="os",
                                            name="ostg")
                            nc.vector.tensor_copy(out=og, in_=wps)
                            # alternate stores between the Pool SWDGE path
                            # and the SP HWDGE ring so neither queue
                            # serializes the output drain; the last bank
                            # goes SWDGE-only so its stores never stall the
                            # SP sequencer ahead of the epilogue transposes
                            if swdge_only or (t + half) % 2 == 0:
                                nc.gpsimd.dma_start(
                                    out=out[ts(t, 128), ts(half, 512)],
                                    in_=og)
                            else:
                                od = nc.sync.dma_start(
                                    out=out[ts(t, 128), ts(half, 512)],
                                    in_=og)
                                add_dep_helper(od.ins, last_dma[0].ins,
                                               sync=False,
                                               reason="odma-order")
                                last_dma[0] = od
                        ops.append((0, None, None, op_stage))
                return ops

            # ---- attention phase B (attn@V flipped + normalize) ----
            EXP_SCALE = (1.0 / (SCALE * SKQ * SKQ) if scores_fp8
                         else 1.0 / SCALE)
            MM_NS = 0.4167    # PE ns per output row at full clock

            def phase_b_ops(lb, g, pps):
                """attn@V + normalize + transpose for wave (lb, g), as
                (est_pe_ns, closure) pairs.  Norm ops for chunk ch are
                issued after chunk ch+1's matmuls so the DVE never parks
                on an attn@V accumulation that hasn't stopped yet."""
                ops = []
                astg = [None]

                def op_astg():
                    astg[0] = astgp.tile([128, 4, 2, DK], BF16, tag="astg",
                                         name="astg")
                ops.append((0, None, None, op_astg))
                paccs = {}

                def chunk_mms(ch):
                    o = []
                    for hh in range(2):
                        def op_alloc(hh=hh, ch=ch):
                            paccs[ch, hh] = psap.tile(
                                [128, 512], F32, tag="psa", name="pacc")
                        o.append((0, None, None, op_alloc))
                    for c in range(LTk):
                        for hh in range(2):
                            def op_mm(c=c, hh=hh, ch=ch, g=g):
                                nc.tensor.matmul(
                                    paccs[ch, hh][:, 0:DK + 1],
                                    lhsT=pps[c][:, hh, ts(ch, 128)],
                                    rhs=v_all[:, c, 2 * g + hh, :],
                                    start=(c == 0), stop=(c == LTk - 1))
                            o.append((65 * MM_NS, None, None, op_mm))
                    return o

                def chunk_norms(ch):
                    o = []
                    for hh in range(2):
                        def op_norm(hh=hh, ch=ch):
                            rc = normp.tile([128, 1], F32, tag="rc")
                            nc.vector.reciprocal(
                                rc, paccs[ch, hh][:, DK:DK + 1])
                            nc.vector.tensor_scalar_mul(
                                astg[0][:, ch, hh, :],
                                paccs[ch, hh][:, 0:DK], rc)
                        o.append((0, None, None, op_norm))
                    return o

                def chunk_tp(ch):
                    # per-chunk transpose: lets the output projection of
                    # Lq-tile lb*4+ch start as soon as this chunk is
                    # normalized instead of after the whole wave
                    def op_tp(ch=ch, lb=lb, g=g):
                        tp = nc.sync.dma_start_transpose(
                            out=ot_sb[:, g, lb * 512 + ch * 128:
                                      lb * 512 + (ch + 1) * 128],
                            in_=astg[0][:, ch, :, :].rearrange(
                                "p a b -> p (a b)"))
                        add_dep_helper(tp.ins, last_dma[0].ins, sync=False,
                                       reason="tp-order")
                        last_dma[0] = tp
                    return [(0, None, None, op_tp)]

                ops += chunk_mms(0)
                ops += chunk_mms(1)
                ops += chunk_norms(0) + chunk_tp(0)
                ops += chunk_mms(2)
                ops += chunk_norms(1) + chunk_tp(1)
                ops += chunk_mms(3)
                ops += chunk_norms(2) + chunk_tp(2)
                ops += chunk_norms(3) + chunk_tp(3)
                return ops

            # ---- phase A for one wave: scores + paired exp ----
            # Background ops are paced by estimated PE cost so the scores
            # feeding the next exp are never buried behind a burst of
            # background matmuls: each c-slot runs its scores first, then
            # ~BG_NS of background work; leftovers carry across waves.
            # fill the exp window (~1038ns) minus this wave's foreground
            # scores cost, with some slack for queue-hop overheads
            BG_NS = 880.0 if scores_fp8 else 740.0

            def run_wave(lb, g, bgq):
                w_abs = lb * 2 + g
                pps = []
                for c in range(LTk):
                    # force-issue overdue background ops (and the FIFO
                    # backlog ahead of them): their consumers are about to
                    # be issued in this slot's foreground
                    last_due = -1
                    for i, (_, due, _, _) in enumerate(bgq):
                        if due is not None and due <= (w_abs, c):
                            last_due = i
                    for _ in range(last_due + 1):
                        _, _, _, op = bgq.pop(0)
                        op()
                    sp = ps2p.tile([128, 2, 512], F32, tag="ps2",
                                   name="spair")
                    for hh in range(2):
                        po = 64 * hh
                        if scores_fp8:
                            nc.tensor.matmul(
                                sp[:, hh, :],
                                lhsT=kt8[po:po + DK, g, c, :, :],
                                rhs=qt8[po:po + DK, g, ts(lb, 512)]
                                .unsqueeze(1).broadcast_to([DK, 2, 512]),
                                start=True, stop=True, perf_mode=DR)
                        else:
                            nc.tensor.matmul(
                                sp[:, hh, :],
                                lhsT=KT[po:po + DK, g, ts(c, 128)],
                                rhs=QT[po:po + DK, g, ts(lb, 512)],
                                start=True, stop=True)
                    pp = ppool.tile([128, 2, 512], BF16, tag="pp",
                                    name="pp")
                    nc.scalar.activation(
                        pp.rearrange("p a b -> p (a b)"),
                        sp.rearrange("p a b -> p (a b)"),
                        mybir.ActivationFunctionType.Exp,
                        scale=EXP_SCALE)
                    pps.append(pp)
                    budget = BG_NS
                    while bgq and budget > 0:
                        cost, _, hold, op = bgq.pop(0)
                        if hold is not None and hold > (w_abs, c):
                            # data very likely not DMA'd yet: issuing now
                            # would park on the PE queue head and stall
                            # everything behind it
                            bgq.insert(0, (cost, None, hold, op))
                            break
                        op()
                        budget -= cost
                return pps

            # ---- prologue: K(block 0 halves), Q(0), V(c 0..1) ----
            for _, _, _, op in kproj_ops(0, groups=[0]):
                op()
            for _, _, _, op in qproj_ops(0, groups=[0]):
                op()

            def spread(ops, w_abs, end_slot, span=3):
                n = len(ops)
                out_ops = []
                for i, (cost, _, hold, op) in enumerate(ops):
                    slot = max(0, end_slot - span + 1 + (i * span) // n - 1)
                    out_ops.append((cost, (w_abs, min(end_slot, slot)),
                                    hold, op))
                return out_ops

            # ---- software pipeline over 8 waves ----
            bgq = []           # carry-over background queue (cost, due, op)
            pps_w = {}
            pend_oproj = []
            for lb in range(LB):
                for g in range(2):
                    if lb == 0 and g == 0:
                        # remaining projections, due-ordered: K tails for
                        # this wave's own scores first, then group-1 K/Q
                        # for wave (0,1), then all of V (consumed by
                        # phase B, FIFO-safe; holds estimate xv arrival)
                        for j in range(1, KB):
                            bgq += spread(kproj_ops(j, groups=[0]),
                                          0, 4 * j, span=2)
                        bgq += spread(kproj_ops(0, groups=[1]), 0, 5,
                                      span=2)
                        for j in range(1, KB):
                            bgq += spread(kproj_ops(j, groups=[1]),
                                          1, 4 * j, span=2)
                        bgq += spread(qproj_ops(0, groups=[1]), 0,
                                      LTk - 1, span=2)
                        bgq += vproj_ops(0, LTk)
                    elif (g == 0 and 1 <= lb < LB - 1) or (lb, g) == (0, 1):
                        # Q proj for bank lb+1, due-staggered
                        bgq += spread(qproj_ops(lb + 1), 2 * lb + g,
                                      LTk - 2, span=6)
                    # o-proj of the previous bank enters the queue after
                    # this wave's Q-projection: its matmuls park on the
                    # transpose-DMA semaphore and would head-of-line block
                    # the PE wait queue right at the bank boundary
                    bgq += pend_oproj
                    pend_oproj = []
                    pps_w[lb, g] = run_wave(lb, g, bgq)
                    bgq += phase_b_ops(lb, g, pps_w[lb, g])
                    if g == 1:
                        pend_oproj = oproj_ops(lb)

            # ---- epilogue: drain remaining background work ----
            for _, _, _, op in bgq + pend_oproj:
                op()

    nc.compile()
    _strip_implied_dma_ring_waits(nc)
    return nc


def _strip_implied_dma_ring_waits(nc):
    """Drop DMA ring-semaphore waits implied by a compute-engine wait on the
    same descriptor (DMA descriptors carry a single hardware sync-wait)."""
    import concourse.mybir as _mb
    for ins in nc.inst_map.values():
        if type(ins).__name__ not in ("InstDMACopy", "InstTensorCopy",
                                      "InstDmaTranspose"):
            continue
        if not ins.outs:
            continue
        memref = getattr(ins.outs[0], "memref", "") or ""
        src_ref = getattr(ins.ins[0], "memref", "") if ins.ins else ""
        if not (memref.startswith(("xb", "ot")) or
                (src_ref or "").startswith(("ostg", "astg"))):
            continue
        si = ins.sync_info
        if not si or not si.on_wait or len(si.on_wait) < 2:
            continue
        eng = [w_ for w_ in si.on_wait
               if not (w_.ant_name or "").startswith(("DMAHW", "DMASW"))]
        if not eng:
            continue
        ins.sync_info = _mb.SyncInfo(on_wait=eng, on_update=list(si.on_update))


def _hilo(x, s):
    """fp8e4m3 hi/lo split of x*s (f32 in, (hi, lo) fp8 out)."""
    import ml_dtypes
    f8 = ml_dtypes.float8_e4m3
    xs = np.asarray(x, np.float32) * s
    hi = xs.astype(f8)
    lo = (xs - hi.astype(np.float32)).astype(f8)
    return hi, lo


def make_in_maps(query, key, value, mask, Wq, bq, Wk, bk, Wv, bv, Wo, bo,
                 LkP, no_bias=True):
    """Host-side sharding + key compaction: per-core input dicts."""
    LTk = LkP // 128
    import ml_dtypes
    bf16 = ml_dtypes.bfloat16
    in_maps = []
    xTs, vos = {}, {}
    for b in range(B):
        keep = np.flatnonzero(~mask[b, 0])
        n = len(keep)
        xkc = np.zeros((D_MODEL, LkP), np.float32)
        xvc = np.zeros((D_MODEL, LkP), np.float32)
        xkc[:, :n] = key[b].T[:, keep]
        xvc[:, :n] = value[b].T[:, keep]
        xTs[b] = {
            "q": _hilo(query[b].T, SX),
            "k": _hilo(xkc, SX),
            "v": _hilo(xvc, SX),
        }
        vo = np.zeros(LkP, np.float32)
        vo[:n] = 1.0
        vos[b] = np.ascontiguousarray(
            vo.reshape(LTk, 128).T.astype(bf16))
    whl = {n: _hilo(W, SW) for n, W in
           (("q", Wq), ("k", Wk), ("v", Wv))}
    for c in range(N_CORES):
        b, hg = divmod(c, N_CORES // B)
        sl = slice(hg * C, (hg + 1) * C)
        m = {"vones": vos[b], "wo": np.ascontiguousarray(
            Wo[sl, :].astype(bf16))}
        for n in ("q", "k", "v"):
            m[f"x{n}h"] = xTs[b][n][0]
            m[f"x{n}l"] = xTs[b][n][1]
            for hl, arr in (("h", whl[n][0]), ("l", whl[n][1])):
                # pack [D, C] -> [128, DC*C] matching the SBUF layout
                wsl = arr[:, sl].reshape(DC, 128, C).transpose(1, 0, 2)
                m[f"w{n}{hl}"] = np.ascontiguousarray(
                    wsl.reshape(128, DC * C))
        if not no_bias:
            m["bq"] = np.ascontiguousarray(bq[sl].astype(np.float32))
            m["bk"] = np.ascontiguousarray(bk[sl].astype(np.float32))
            m["bvr"] = np.ascontiguousarray(
                (bv[sl].astype(np.float32) / PS_SCALE).astype(bf16)[None, :])
        in_maps.append(m)
    return in_maps


_NC_CACHE = {}


def _get_nc(LkP, no_bias=True):
    key = (LkP, no_bias)
    if key not in _NC_CACHE:
        _NC_CACHE[key] = build_nc(LkP, no_bias=no_bias)
    return _NC_CACHE[key]


def run(inputs, trace=False):
    """Run on 8 cores; returns (full_output, BassKernelResults)."""
    inputs = {k: np.asarray(v) for k, v in inputs.items()}
    mask = inputs["mask"]
    counts = [int((~mask[b, 0]).sum()) for b in range(B)]
    LkP = max(128, 128 * int(math.ceil(max(counts) / 128.0)))
    no_bias = not (np.any(inputs["bq"]) or np.any(inputs["bk"])
                   or np.any(inputs["bv"]))
    nc = _get_nc(LkP, no_bias)
    in_maps = make_in_maps(**inputs, LkP=LkP, no_bias=no_bias)
    res = run_bass_kernel_spmd(nc, in_maps, list(range(N_CORES)), trace=trace)
    groups_per_batch = N_CORES // B
    out = np.zeros((B, L, D_MODEL), np.float32)
    for b in range(B):
        acc = np.zeros((L, D_MODEL), np.float32)
        if counts[b] > 0:
            for hg in range(groups_per_batch):
                acc += np.asarray(
                    res.results[b * groups_per_batch + hg]["partial"]
                ).astype(np.float32)
        out[b] = acc + inputs["bo"][None, :]
    return out, res


def kernel(**inputs) -> np.ndarray:
    out, _ = run(inputs)
    return out
